# revision 1
# baseline (speedup 1.0000x reference)
"""DeltaSynapse kernel for Trainium2 (8 NeuronCores, SPMD).

Reference computation:
    Xpre[b,e,o] = sum_d delaymap[d,e,o] * Xd[d,b,e]
    I[b,o]      = sum_e (signs*W)[e,o] * Xpre[b,e,o]

Folded:  I[b,o] = sum_{d,e} (delaymap[d,e,o] * Weff[e,o]) * Xd[d,b,e]
i.e. a sum of D matmuls  I += Xd[d] @ (delaymap[d] . Weff).

Sharding: shard the contraction (pre-neuron e) dim across the 8 cores
(256 rows each). Each core reads its own e-slice of delaymap/W/signs/Xd
(~20.1 MiB of fp32 HBM reads, nothing replicated) and produces a full
[16, 2048] partial output; the host sums the 8 partials. Memory-bound:
roofline ~ 20 MiB / ~430 GB/s.

On-chip dtype: fp16. delaymap is one-hot (0/1 -> exact in fp16); W/Xd
lose only 2^-11 rel. SWDGE DMA casts fp32->fp16 in the datapath, so
HBM reads stay fp32 (full bytes) while SBUF tiles halve and the DVE
multiply runs in 2x mode. PE runs fp16 at full rate (1 cyc/row).

Pipeline: delaymap streams in (o-range, e-chunk) slabs, o-major, so
each o-range's 16-matmul PSUM accumulation finishes as soon as its
last slab lands and its output DMA overlaps the remaining stream. The
final o-ranges are half-width so the post-DMA tail is short.
"""

import numpy as np

D, B, N = 8, 16, 2048
NCORES = 8
P = 128                 # SBUF partitions / matmul contraction tile
ESH = N // NCORES       # per-core pre-dim shard = 256
ECH = ESH // P          # e-chunks per core = 2
# output o-ranges: full-width blocks first, narrow at the end so the
# post-DMA elementwise+matmul+output tail is short
O_RANGES = [
    (0, 512),
    (512, 1024),
    (1024, 1536),
    (1536, 1792),
    (1792, 1920),
    (1920, 1984),
    (1984, 2048),
]
# delaymap slabs: one per (o-range, e-chunk), issued o-major
SLABS = [(r, c) for r in range(len(O_RANGES)) for c in range(ECH)]

_prog_cache = {}


def _build_program():
    from concourse import bacc, tile
    from concourse import mybir

    f32 = mybir.dt.float32
    f16 = mybir.dt.float16

    nc = bacc.Bacc(num_swdge_queues=2)
    # Host-prepared layouts (see kernel() below), all fp32 in HBM:
    #   dm{r}_{c}: [P, D, len_r]   delaymap[d, c*128+p, o_range r]
    #   wsa : [P, 2, N]            W/signs rows for e-chunk 0
    #   wsb : [P, 2, N]            W/signs rows for e-chunk 1
    #   xd  : [P, ECH, D, B]       Xd slice transposed
    dms = {}
    for r, c in SLABS:
        o0, o1 = O_RANGES[r]
        dms[(r, c)] = nc.dram_tensor(
            f"dm{r}_{c}", [P, D, o1 - o0], f32, kind="ExternalInput"
        )
    wsa = nc.dram_tensor("wsa", [P, 2, N], f32, kind="ExternalInput")
    wsb = nc.dram_tensor("wsb", [P, 2, N], f32, kind="ExternalInput")
    xd = nc.dram_tensor("xd", [P, ECH, D, B], f32, kind="ExternalInput")
    out = nc.dram_tensor("out", [B, N], f32, kind="ExternalOutput")

    with tile.TileContext(nc) as tc:
        with (
            tc.tile_pool(name="const", bufs=1) as cpool,
            tc.tile_pool(name="dm", bufs=6) as dmpool,
            tc.tile_pool(name="wd", bufs=6) as wdpool,
            tc.tile_pool(name="psum", bufs=1, space="PSUM") as ppool,
            tc.tile_pool(name="outp", bufs=6) as opool,
        ):
            wsa_t = cpool.tile([P, 2, N], f16)
            wsb_t = cpool.tile([P, 2, N], f16)
            weff = cpool.tile([P, ECH, N], f16)
            xd_h = cpool.tile([P, ECH, D, B], f16)

            dm_tiles = {}
            for r, c in SLABS:
                o0, o1 = O_RANGES[r]
                dm_tiles[(r, c)] = dmpool.tile(
                    [P, D, o1 - o0], f16, tag="dmslab", name=f"dm{r}_{c}"
                )

            # SWDGE (gpsimd) DMAs cast fp32->fp16 in the datapath.
            order = [SLABS[0], "wsa", "xd", SLABS[1], "wsb"] + SLABS[2:]
            for item in order:
                if item == "wsa":
                    nc.gpsimd.dma_start(wsa_t[:], wsa[:])
                elif item == "wsb":
                    nc.gpsimd.dma_start(wsb_t[:], wsb[:])
                elif item == "xd":
                    nc.gpsimd.dma_start(xd_h[:], xd[:])
                else:
                    nc.gpsimd.dma_start(dm_tiles[item][:], dms[item][:])

            nc.vector.tensor_mul(weff[:, 0, :], wsa_t[:, 0], wsa_t[:, 1])
            nc.vector.tensor_mul(weff[:, 1, :], wsb_t[:, 0], wsb_t[:, 1])

            psum = ppool.tile([B, N], f32)
            for si, (r, c) in enumerate(SLABS):
                o0, o1 = O_RANGES[r]
                dm_t = dm_tiles[(r, c)]
                wd_t = wdpool.tile([P, D, o1 - o0], f16, tag="wd")
                nc.vector.tensor_mul(
                    wd_t[:],
                    dm_t[:],
                    weff[:, c, o0:o1].unsqueeze(1).broadcast_to(
                        [P, D, o1 - o0]
                    ),
                )
                for d in range(D):
                    nc.tensor.matmul(
                        psum[:, o0:o1],
                        xd_h[:, c, d, :],
                        wd_t[:, d, :],
                        start=(c == 0 and d == 0),
                        stop=(c == ECH - 1 and d == D - 1),
                    )
                # o-range r complete after its last e-chunk: stream it out
                if c == ECH - 1:
                    out_t = opool.tile([B, o1 - o0], f32, tag="out", name=f"o{r}")
                    nc.scalar.copy(out_t[:], psum[:, o0:o1])
                    nc.sync.dma_start(out[:, o0:o1], out_t[:])

    nc.compile()
    return nc


def _get_program():
    if "nc" not in _prog_cache:
        _prog_cache["nc"] = _build_program()
    return _prog_cache["nc"]


def _shard_inputs(Xd, delaymap, W, signs):
    """Pure layout permutation/slicing -> per-core input maps."""
    Xd = np.ascontiguousarray(np.asarray(Xd, dtype=np.float32))
    delaymap = np.asarray(delaymap, dtype=np.float32)
    W = np.asarray(W, dtype=np.float32)
    signs = np.asarray(signs, dtype=np.float32)

    in_maps = []
    for k in range(NCORES):
        esl = slice(k * ESH, (k + 1) * ESH)
        # delaymap [D, ESH, N] -> per-chunk [c][P, D, N], then o-sliced
        dm_cpd = delaymap[:, esl, :].reshape(D, ECH, P, N).transpose(1, 2, 0, 3)
        m = {}
        for r, c in SLABS:
            o0, o1 = O_RANGES[r]
            m[f"dm{r}_{c}"] = np.ascontiguousarray(dm_cpd[c, :, :, o0:o1])
        # W/signs rows for this core's e-slice -> per-chunk [P, 2, N]
        wk = W[esl].reshape(ECH, P, N)
        sk = signs[esl].reshape(ECH, P, N)
        m["wsa"] = np.ascontiguousarray(np.stack([wk[0], sk[0]], axis=1))
        m["wsb"] = np.ascontiguousarray(np.stack([wk[1], sk[1]], axis=1))
        # Xd [D, B, ESH] -> [P, ECH, D, B]
        m["xd"] = np.ascontiguousarray(
            Xd[:, :, esl].reshape(D, B, ECH, P).transpose(3, 2, 0, 1)
        )
        in_maps.append(m)
    return in_maps


def _run(in_maps, trace=False, **kw):
    from concourse.bass_utils import run_bass_kernel_spmd

    nc = _get_program()
    return run_bass_kernel_spmd(nc, in_maps, list(range(NCORES)), trace=trace, **kw)


def _gather(res):
    acc = np.zeros((B, N), dtype=np.float64)
    for k in range(NCORES):
        acc += res.results[k]["out"].astype(np.float64)
    return acc.astype(np.float32)


def kernel(Xd, X, delaymap, W, signs):
    in_maps = _shard_inputs(Xd, delaymap, W, signs)
    return _gather(_run(in_maps))



# revision 3
# speedup vs baseline: 1.4401x; 1.4401x over previous
"""DeltaSynapse kernel for Trainium2 (8 NeuronCores, SPMD).

Reference computation:
    Xpre[b,e,o] = sum_d delaymap[d,e,o] * Xd[d,b,e]
    I[b,o]      = sum_e (signs*W)[e,o] * Xpre[b,e,o]

Folded:  I[b,o] = sum_{d,e} (delaymap[d,e,o] * Weff[e,o]) * Xd[d,b,e]
i.e. a sum of D matmuls  I += Xd[d] @ (delaymap[d] . Weff).

Sharding: shard the contraction (pre-neuron e) dim across the 8 cores
(256 rows each). Each core reads its own e-slice of delaymap/W/signs/Xd
and produces a full [16, 2048] partial output; the host sums the 8
partials.

v2 vs v1: host shards are laid out in fp16 (delaymap is one-hot 0/1 ->
exact in fp16; W/signs/Xd lose 2^-11 rel). HBM reads halve to ~10 MiB
per core and all input DMAs become plain HWDGE (sync/scalar engines) --
no SWDGE cast datapath, gpsimd idle. Matmul restructure: 5 o-ranges
(1024/512/256/128/128) instead of 7, so the PE does 80 LDWEIGHTS
instead of 112 and streams wider moving operands. The o-range taper
keeps the post-stream tail (TT+MM+copy+DMA of the last range) short.
"""

import numpy as np

D, B, N = 8, 16, 2048
NCORES = 8
P = 128                 # SBUF partitions / matmul contraction tile
ESH = N // NCORES       # per-core pre-dim shard = 256
ECH = ESH // P          # e-chunks per core = 2

# DMA slabs of delaymap (o-extent, c-extent). Slab 0 is split per
# e-chunk so the first TT can start after ~2.1 MiB instead of 4.2.
# Compute ranges (o0, o1, slab, slab-local o0): taper toward the end.
DMA_SLABS = [
    ("dm0a", 0, 1024, 0),     # name, o0, o1, single chunk c=0
    ("dm0b", 0, 1024, 1),     # c=1
    ("dm1", 1024, 1536, None),  # both chunks interleaved [P,ECH,D,olen]
    ("dm2", 1536, 1792, None),
    ("dm3", 1792, 2048, None),
]
C_RANGES = [
    # (o0, o1, slab names per chunk, local offset)
    (0, 1024, ("dm0a", "dm0b"), 0),
    (1024, 1536, ("dm1", "dm1"), 0),
    (1536, 1792, ("dm2", "dm2"), 0),
    (1792, 1920, ("dm3", "dm3"), 0),
    (1920, 2048, ("dm3", "dm3"), 128),
]

_prog_cache = {}


def _build_program():
    from concourse import bacc, tile
    from concourse import mybir

    f32 = mybir.dt.float32
    f16 = mybir.dt.float16

    nc = bacc.Bacc()
    # Host-prepared layouts (see kernel() below), all fp16 in HBM:
    #   dm0a/dm0b: [P, D, 1024]       delaymap[d, e-chunk c, o 0:1024]
    #   dm1..3   : [P, ECH, D, olen]  delaymap o-slabs, both chunks
    #   ws  : [P, ECH, 2, N]          (W, signs) rows per e-chunk
    #   xd  : [P, ECH, D, B]          Xd slice transposed
    dram = {}
    for name, o0, o1, c in DMA_SLABS:
        shape = [P, D, o1 - o0] if c is not None else [P, ECH, D, o1 - o0]
        dram[name] = nc.dram_tensor(name, shape, f16, kind="ExternalInput")
    ws = nc.dram_tensor("ws", [P, ECH, 2, N], f16, kind="ExternalInput")
    xd = nc.dram_tensor("xd", [P, ECH, D, B], f16, kind="ExternalInput")
    out = nc.dram_tensor("out", [B, N], f32, kind="ExternalOutput")

    with tile.TileContext(nc) as tc:
        with (
            tc.tile_pool(name="const", bufs=1) as cpool,
            tc.tile_pool(name="dm", bufs=3) as dmpool,
            tc.tile_pool(name="wd", bufs=4) as wdpool,
            tc.tile_pool(name="psum", bufs=1, space="PSUM") as ppool,
            tc.tile_pool(name="outp", bufs=5) as opool,
        ):
            ws_t = cpool.tile([P, ECH, 2, N], f16)
            weff = cpool.tile([P, ECH, N], f16)
            xd_t = cpool.tile([P, ECH, D, B], f16)

            dm_tiles = {}
            for name, o0, o1, c in DMA_SLABS:
                shape = [P, D, o1 - o0] if c is not None else [P, ECH, D, o1 - o0]
                dm_tiles[name] = dmpool.tile(shape, f16, tag="dmslab", name=name)

            # Input DMAs, all HWDGE. ws/xd on scalar; dm stream on sync,
            # o-major so compute starts as soon as slab 0 lands.
            nc.scalar.dma_start(ws_t[:], ws[:])
            nc.scalar.dma_start(xd_t[:], xd[:])
            for name, _, _, _ in DMA_SLABS:
                nc.sync.dma_start(dm_tiles[name][:], dram[name][:])

            # Weff = W * signs per e-chunk (DVE, ahead of the slab TTs).
            nc.vector.tensor_mul(weff[:, 0, :], ws_t[:, 0, 0], ws_t[:, 0, 1])
            nc.vector.tensor_mul(weff[:, 1, :], ws_t[:, 1, 0], ws_t[:, 1, 1])

            psum = ppool.tile([B, N], f32)
            for o0, o1, slabs, loc in C_RANGES:
                olen = o1 - o0
                wd_ts = []
                for c in range(ECH):
                    dm_t = dm_tiles[slabs[c]]
                    src = dm_t[:, :, loc:loc + olen] if dm_t.shape[1] == D \
                        else dm_t[:, c, :, loc:loc + olen]
                    wd_t = wdpool.tile([P, D, olen], f16, tag="wd")
                    nc.vector.tensor_mul(
                        wd_t[:],
                        src,
                        weff[:, c, o0:o1].unsqueeze(1).broadcast_to(
                            [P, D, olen]
                        ),
                    )
                    wd_ts.append(wd_t)
                nmm = (olen + 511) // 512  # ISA moving-operand cap
                for c in range(ECH):
                    for d in range(D):
                        for j in range(nmm):
                            j0, j1 = j * 512, min((j + 1) * 512, olen)
                            nc.tensor.matmul(
                                psum[:, o0 + j0:o0 + j1],
                                xd_t[:, c, d, :],
                                wd_ts[c][:, d, j0:j1],
                                start=(c == 0 and d == 0),
                                stop=(c == ECH - 1 and d == D - 1),
                            )
                out_t = opool.tile([B, olen], f32, tag="out", name=f"o{o0}")
                nc.scalar.copy(out_t[:], psum[:, o0:o1])
                nc.sync.dma_start(out[:, o0:o1], out_t[:])

    nc.compile()
    return nc


def _get_program():
    if "nc" not in _prog_cache:
        _prog_cache["nc"] = _build_program()
    return _prog_cache["nc"]


def _shard_inputs(Xd, delaymap, W, signs):
    """Layout permutation/slicing + fp16 cast -> per-core input maps."""
    Xd = np.asarray(Xd, dtype=np.float32)
    delaymap = np.asarray(delaymap, dtype=np.float32)
    W = np.asarray(W, dtype=np.float32)
    signs = np.asarray(signs, dtype=np.float32)

    in_maps = []
    for k in range(NCORES):
        esl = slice(k * ESH, (k + 1) * ESH)
        # delaymap [D, ESH, N] -> [ECH, P, D, N] fp16
        dm_cpd = (
            delaymap[:, esl, :]
            .reshape(D, ECH, P, N)
            .transpose(1, 2, 0, 3)
            .astype(np.float16)
        )
        m = {}
        for name, o0, o1, c in DMA_SLABS:
            if c is not None:
                m[name] = np.ascontiguousarray(dm_cpd[c, :, :, o0:o1])
            else:
                # [P, ECH, D, olen]
                m[name] = np.ascontiguousarray(
                    dm_cpd[:, :, :, o0:o1].transpose(1, 0, 2, 3)
                )
        # W/signs rows for this core's e-slice -> [P, ECH, 2, N] fp16
        wk = W[esl].reshape(ECH, P, N).astype(np.float16)
        sk = signs[esl].reshape(ECH, P, N).astype(np.float16)
        m["ws"] = np.ascontiguousarray(
            np.stack([wk, sk], axis=2).transpose(1, 0, 2, 3)
        )
        # Xd [D, B, ESH] -> [P, ECH, D, B] fp16
        m["xd"] = np.ascontiguousarray(
            Xd[:, :, esl].reshape(D, B, ECH, P).transpose(3, 2, 0, 1)
        ).astype(np.float16)
        in_maps.append(m)
    return in_maps


def _run(in_maps, trace=False, **kw):
    from concourse.bass_utils import run_bass_kernel_spmd

    nc = _get_program()
    return run_bass_kernel_spmd(nc, in_maps, list(range(NCORES)), trace=trace, **kw)


def _gather(res):
    acc = np.zeros((B, N), dtype=np.float64)
    for k in range(NCORES):
        acc += res.results[k]["out"].astype(np.float64)
    return acc.astype(np.float32)


def kernel(Xd, X, delaymap, W, signs):
    in_maps = _shard_inputs(Xd, delaymap, W, signs)
    return _gather(_run(in_maps))


# revision 4
# speedup vs baseline: 1.5646x; 1.0865x over previous
"""DeltaSynapse kernel for Trainium2 (8 NeuronCores, SPMD).

Reference computation:
    Xpre[b,e,o] = sum_d delaymap[d,e,o] * Xd[d,b,e]
    I[b,o]      = sum_e (signs*W)[e,o] * Xpre[b,e,o]

Folded:  I[b,o] = sum_{d,e} (delaymap[d,e,o] * Weff[e,o]) * Xd[d,b,e]
i.e. a sum of D matmuls  I += Xd[d] @ (delaymap[d] . Weff).

Sharding: shard the contraction (pre-neuron e) dim across the 8 cores
(256 rows each). Each core reads its own e-slice of delaymap/W/signs/Xd
and produces a full [16, 2048] partial output; the host sums the 8
partials.

v3: host shards are fp16 (delaymap one-hot 0/1 -> exact in fp16), all
input DMAs are plain HWDGE on the sync queue in dependency order
(ws -> xd -> delaymap o-major) so the Weff multiply and first slab
multiply start as early as possible. The dm*weff elementwise work is
split DVE (d=0..6) / gpsimd (d=7) since gpsimd is otherwise idle.
Scalar engine only does PSUM->SBUF copies and output DMAs. o-ranges
taper so the post-stream tail is short.
"""

import numpy as np

D, B, N = 8, 16, 2048
NCORES = 8
P = 128                 # SBUF partitions / matmul contraction tile
ESH = N // NCORES       # per-core pre-dim shard = 256
ECH = ESH // P          # e-chunks per core = 2
DV = 7                  # d-slots 0..6 multiplied on DVE, slot 7 on gpsimd

# DMA slabs of delaymap: (name, o0, o1); tile [P, ECH, D, olen].
DMA_SLABS = [
    ("dm0", 0, 512),
    ("dm1", 512, 1024),
    ("dm2", 1024, 1536),
    ("dm3", 1536, 1792),
    ("dm4", 1792, 2048),
]
# compute ranges (o0, o1, slab idx, slab-local offset): taper at the end
C_RANGES = [
    (0, 512, 0, 0),
    (512, 1024, 1, 0),
    (1024, 1536, 2, 0),
    (1536, 1792, 3, 0),
    (1792, 1920, 4, 0),
    (1920, 1984, 4, 128),
    (1984, 2048, 4, 192),
]

_prog_cache = {}


def _build_program():
    from concourse import bacc, tile
    from concourse import mybir

    f32 = mybir.dt.float32
    f16 = mybir.dt.float16

    nc = bacc.Bacc()
    # Host-prepared layouts (see kernel() below), all fp16 in HBM:
    #   dm{i}: [P, ECH, D, olen]  delaymap o-slab, both e-chunks
    #   ws   : [P, ECH, 2, N]     (W, signs) rows per e-chunk
    #   xd   : [P, ECH, D, B]     Xd slice transposed
    dram = {}
    for name, o0, o1 in DMA_SLABS:
        dram[name] = nc.dram_tensor(name, [P, ECH, D, o1 - o0], f16,
                                    kind="ExternalInput")
    ws = nc.dram_tensor("ws", [P, ECH, 2, N], f16, kind="ExternalInput")
    xd = nc.dram_tensor("xd", [P, ECH, D, B], f16, kind="ExternalInput")
    out = nc.dram_tensor("out", [B, N], f32, kind="ExternalOutput")

    with tile.TileContext(nc) as tc:
        with (
            tc.tile_pool(name="const", bufs=1) as cpool,
            tc.tile_pool(name="dm", bufs=3) as dmpool,
            tc.tile_pool(name="wd", bufs=6) as wdpool,
            tc.tile_pool(name="psum", bufs=1, space="PSUM") as ppool,
            tc.tile_pool(name="outp", bufs=7) as opool,
        ):
            ws_t = cpool.tile([P, ECH, 2, N], f16)
            weff = cpool.tile([P, ECH, N], f16)
            xd_t = cpool.tile([P, ECH, D, B], f16)

            dm_tiles = []
            for name, o0, o1 in DMA_SLABS:
                dm_tiles.append(
                    dmpool.tile([P, ECH, D, o1 - o0], f16, tag="dmslab",
                                name=name)
                )

            # All input DMAs HWDGE on the sync queue, dependency order:
            # ws gates weff; dm slabs stream o-major behind it.
            nc.sync.dma_start(ws_t[:], ws[:])
            nc.sync.dma_start(xd_t[:], xd[:])
            for t, (name, _, _) in zip(dm_tiles, DMA_SLABS):
                nc.sync.dma_start(t[:], dram[name][:])

            # Weff = W * signs per e-chunk (DVE, head of the chain).
            nc.vector.tensor_mul(weff[:, 0, :], ws_t[:, 0, 0], ws_t[:, 0, 1])
            nc.vector.tensor_mul(weff[:, 1, :], ws_t[:, 1, 0], ws_t[:, 1, 1])

            psum = ppool.tile([B, N], f32)
            for o0, o1, si, loc in C_RANGES:
                olen = o1 - o0
                dm_t = dm_tiles[si]
                wd_ts = []
                for c in range(ECH):
                    src = dm_t[:, c, :, loc:loc + olen]
                    wd_t = wdpool.tile([P, D, olen], f16, tag="wd")
                    wb = weff[:, c, o0:o1].unsqueeze(1)
                    # d-slots 0..6 on DVE, slot 7 on gpsimd (else idle)
                    nc.vector.tensor_mul(
                        wd_t[:, :DV, :], src[:, :DV, :],
                        wb.broadcast_to([P, DV, olen]),
                    )
                    nc.gpsimd.tensor_mul(
                        wd_t[:, DV:, :], src[:, DV:, :],
                        wb.broadcast_to([P, D - DV, olen]),
                    )
                    wd_ts.append(wd_t)
                for c in range(ECH):
                    for d in range(D):
                        nc.tensor.matmul(
                            psum[:, o0:o1],
                            xd_t[:, c, d, :],
                            wd_ts[c][:, d, :],
                            start=(c == 0 and d == 0),
                            stop=(c == ECH - 1 and d == D - 1),
                        )
                out_t = opool.tile([B, olen], f32, tag="out", name=f"o{o0}")
                nc.scalar.copy(out_t[:], psum[:, o0:o1])
                nc.scalar.dma_start(out[:, o0:o1], out_t[:])

    nc.compile()
    return nc


def _get_program():
    if "nc" not in _prog_cache:
        _prog_cache["nc"] = _build_program()
    return _prog_cache["nc"]


def _shard_inputs(Xd, delaymap, W, signs):
    """Layout permutation/slicing + fp16 cast -> per-core input maps."""
    Xd = np.asarray(Xd, dtype=np.float32)
    delaymap = np.asarray(delaymap, dtype=np.float32)
    W = np.asarray(W, dtype=np.float32)
    signs = np.asarray(signs, dtype=np.float32)

    in_maps = []
    for k in range(NCORES):
        esl = slice(k * ESH, (k + 1) * ESH)
        # delaymap [D, ESH, N] -> [P, ECH, D, N] fp16
        dm_pcd = (
            delaymap[:, esl, :]
            .reshape(D, ECH, P, N)
            .transpose(2, 1, 0, 3)
            .astype(np.float16)
        )
        m = {}
        for name, o0, o1 in DMA_SLABS:
            m[name] = np.ascontiguousarray(dm_pcd[:, :, :, o0:o1])
        # W/signs rows for this core's e-slice -> [P, ECH, 2, N] fp16
        wk = W[esl].reshape(ECH, P, N).astype(np.float16)
        sk = signs[esl].reshape(ECH, P, N).astype(np.float16)
        m["ws"] = np.ascontiguousarray(
            np.stack([wk, sk], axis=2).transpose(1, 0, 2, 3)
        )
        # Xd [D, B, ESH] -> [P, ECH, D, B] fp16
        m["xd"] = np.ascontiguousarray(
            Xd[:, :, esl].reshape(D, B, ECH, P).transpose(3, 2, 0, 1)
        ).astype(np.float16)
        in_maps.append(m)
    return in_maps


def _run(in_maps, trace=False, **kw):
    from concourse.bass_utils import run_bass_kernel_spmd

    nc = _get_program()
    return run_bass_kernel_spmd(nc, in_maps, list(range(NCORES)), trace=trace, **kw)


def _gather(res):
    acc = np.zeros((B, N), dtype=np.float64)
    for k in range(NCORES):
        acc += res.results[k]["out"].astype(np.float64)
    return acc.astype(np.float32)


def kernel(Xd, X, delaymap, W, signs):
    in_maps = _shard_inputs(Xd, delaymap, W, signs)
    return _gather(_run(in_maps))


# revision 5
# speedup vs baseline: 1.6551x; 1.0578x over previous
"""DeltaSynapse kernel for Trainium2 (8 NeuronCores, SPMD).

Reference computation:
    Xpre[b,e,o] = sum_d delaymap[d,e,o] * Xd[d,b,e]
    I[b,o]      = sum_e (signs*W)[e,o] * Xpre[b,e,o]

Folded:  I[b,o] = sum_{d,e} (delaymap[d,e,o] * Weff[e,o]) * Xd[d,b,e]
i.e. a sum of D matmuls  I += Xd[d] @ (delaymap[d] . Weff).

Sharding: shard the contraction (pre-neuron e) dim across the 8 cores
(256 rows each). Each core reads its own e-slice of delaymap/W/signs/Xd
and produces a full [16, 2048] partial output; the host sums the 8
partials.

v4: fp16 host shards, all input DMAs HWDGE on the sync queue. Critical
chain = DVE tensor_tensor stream (dm*weff, ~19 us): the head is
shortened by splitting W/signs per e-chunk so both weff multiplies
complete while slab 0 is still in flight, and every TT reads a flat
[P, D, w] region (v3's d-sliced APs ran ~40% slower). Tail ranges
(128/128) live in one DMA slab but as separately-stacked flat blocks.
PSUM copies + output DMAs on the scalar engine; gpsimd idle (its
tensor ops measured ~6x slower than modeled).
"""

import numpy as np

D, B, N = 8, 16, 2048
NCORES = 8
P = 128                 # SBUF partitions / matmul contraction tile
ESH = N // NCORES       # per-core pre-dim shard = 256
ECH = ESH // P          # e-chunks per core = 2

# DMA slabs: (name, [list of o-ranges]); each slab tile is
# [P, NR, ECH, D, w] with equal-width ranges stacked flat.
DMA_SLABS = [
    ("dm0", [(0, 512)]),
    ("dm1", [(512, 1024)]),
    ("dm2", [(1024, 1536)]),
    ("dm3", [(1536, 1792)]),
    ("dm4", [(1792, 1920), (1920, 2048)]),
]

_prog_cache = {}


def _build_program():
    from concourse import bacc, tile
    from concourse import mybir

    f32 = mybir.dt.float32
    f16 = mybir.dt.float16

    nc = bacc.Bacc()
    # Host-prepared layouts (see kernel() below), all fp16 in HBM:
    #   dm{i}: [P, NR, ECH, D, w]  delaymap o-slab (flat per range/chunk)
    #   wsa/wsb: [P, 2, N]         (W, signs) rows, e-chunk a/b
    #   xd   : [P, ECH, D, B]      Xd slice transposed
    dram = {}
    for name, ranges in DMA_SLABS:
        w = ranges[0][1] - ranges[0][0]
        dram[name] = nc.dram_tensor(
            name, [P, len(ranges), ECH, D, w], f16, kind="ExternalInput"
        )
    wsa = nc.dram_tensor("wsa", [P, 2, N], f16, kind="ExternalInput")
    wsb = nc.dram_tensor("wsb", [P, 2, N], f16, kind="ExternalInput")
    xd = nc.dram_tensor("xd", [P, ECH, D, B], f16, kind="ExternalInput")
    out = nc.dram_tensor("out", [B, N], f32, kind="ExternalOutput")

    with tile.TileContext(nc) as tc:
        with (
            tc.tile_pool(name="const", bufs=1) as cpool,
            tc.tile_pool(name="dm", bufs=3) as dmpool,
            tc.tile_pool(name="wd", bufs=6) as wdpool,
            tc.tile_pool(name="psum", bufs=1, space="PSUM") as ppool,
            tc.tile_pool(name="outp", bufs=6) as opool,
        ):
            wsa_t = cpool.tile([P, 2, N], f16)
            wsb_t = cpool.tile([P, 2, N], f16)
            weff = cpool.tile([P, ECH, N], f16)
            xd_t = cpool.tile([P, ECH, D, B], f16)

            dm_tiles = {}
            for name, ranges in DMA_SLABS:
                w = ranges[0][1] - ranges[0][0]
                dm_tiles[name] = dmpool.tile(
                    [P, len(ranges), ECH, D, w], f16, tag="dmslab", name=name
                )

            # All input DMAs HWDGE on the sync queue, dependency order:
            # both ws chunks land before slab 0 so weff never stalls DVE.
            nc.sync.dma_start(wsa_t[:], wsa[:])
            nc.sync.dma_start(wsb_t[:], wsb[:])
            nc.sync.dma_start(xd_t[:], xd[:])
            for name, _ in DMA_SLABS:
                nc.sync.dma_start(dm_tiles[name][:], dram[name][:])

            # Weff = W * signs per e-chunk (DVE, head of the chain).
            nc.vector.tensor_mul(weff[:, 0, :], wsa_t[:, 0], wsa_t[:, 1])
            nc.vector.tensor_mul(weff[:, 1, :], wsb_t[:, 0], wsb_t[:, 1])

            psum = ppool.tile([B, N], f32)
            for name, ranges in DMA_SLABS:
                dm_t = dm_tiles[name]
                for r, (o0, o1) in enumerate(ranges):
                    olen = o1 - o0
                    wd_ts = []
                    for c in range(ECH):
                        wd_t = wdpool.tile([P, D, olen], f16, tag="wd")
                        nc.vector.tensor_mul(
                            wd_t[:],
                            dm_t[:, r, c],
                            weff[:, c, o0:o1].unsqueeze(1).broadcast_to(
                                [P, D, olen]
                            ),
                        )
                        wd_ts.append(wd_t)
                    for c in range(ECH):
                        for d in range(D):
                            nc.tensor.matmul(
                                psum[:, o0:o1],
                                xd_t[:, c, d, :],
                                wd_ts[c][:, d, :],
                                start=(c == 0 and d == 0),
                                stop=(c == ECH - 1 and d == D - 1),
                            )
                    out_t = opool.tile([B, olen], f32, tag="out",
                                       name=f"o{o0}")
                    nc.scalar.copy(out_t[:], psum[:, o0:o1])
                    nc.scalar.dma_start(out[:, o0:o1], out_t[:])

    nc.compile()
    return nc


def _get_program():
    if "nc" not in _prog_cache:
        _prog_cache["nc"] = _build_program()
    return _prog_cache["nc"]


def _shard_inputs(Xd, delaymap, W, signs):
    """Layout permutation/slicing + fp16 cast -> per-core input maps."""
    Xd = np.asarray(Xd, dtype=np.float32)
    delaymap = np.asarray(delaymap, dtype=np.float32)
    W = np.asarray(W, dtype=np.float32)
    signs = np.asarray(signs, dtype=np.float32)

    in_maps = []
    for k in range(NCORES):
        esl = slice(k * ESH, (k + 1) * ESH)
        # delaymap [D, ESH, N] -> [P, ECH, D, N] fp16
        dm_pcd = (
            delaymap[:, esl, :]
            .reshape(D, ECH, P, N)
            .transpose(2, 1, 0, 3)
            .astype(np.float16)
        )
        m = {}
        for name, ranges in DMA_SLABS:
            # [P, NR, ECH, D, w]
            m[name] = np.ascontiguousarray(
                np.stack([dm_pcd[:, :, :, o0:o1] for o0, o1 in ranges],
                         axis=1)
            )
        # W/signs rows per e-chunk -> [P, 2, N] fp16 each
        wk = W[esl].reshape(ECH, P, N).astype(np.float16)
        sk = signs[esl].reshape(ECH, P, N).astype(np.float16)
        m["wsa"] = np.ascontiguousarray(np.stack([wk[0], sk[0]], axis=1))
        m["wsb"] = np.ascontiguousarray(np.stack([wk[1], sk[1]], axis=1))
        # Xd [D, B, ESH] -> [P, ECH, D, B] fp16
        m["xd"] = np.ascontiguousarray(
            Xd[:, :, esl].reshape(D, B, ECH, P).transpose(3, 2, 0, 1)
        ).astype(np.float16)
        in_maps.append(m)
    return in_maps


def _run(in_maps, trace=False, **kw):
    from concourse.bass_utils import run_bass_kernel_spmd

    nc = _get_program()
    return run_bass_kernel_spmd(nc, in_maps, list(range(NCORES)), trace=trace, **kw)


def _gather(res):
    acc = np.zeros((B, N), dtype=np.float64)
    for k in range(NCORES):
        acc += res.results[k]["out"].astype(np.float64)
    return acc.astype(np.float32)


def kernel(Xd, X, delaymap, W, signs):
    in_maps = _shard_inputs(Xd, delaymap, W, signs)
    return _gather(_run(in_maps))
